# revision 1
# baseline (speedup 1.0000x reference)
"""Trainium2 Bass kernel for nn_ApproxSymmetricNet (gnn_message_passing).

8 NeuronCores. chi is sharded by SITE (each core computes 1/8 of the sites
for all 128 batches — this cuts the SWDGE descriptor-generation work, which
is the kernel's bottleneck, 8x for that stage), then an AllToAll converts
h1 to BATCH-sharded form; wilson and omega run data-parallel over batch
(16 batches/core) with all gathers local.

Gathers use the SWDGE dma_gather ucode (one descriptor per gathered row,
~9ns/descriptor of Q7 time — the governing cost). Rows are 512B.

Layouts (f32 everywhere):
  h1/h2 rows: [ri(2), i(4), b(16)] = 128 floats = 512B.
  chi:   gather x_full rows (site -> 128 batches) in a (site-group x tap)
         partition layout; one PE matmul per 56-site chunk against a
         block-diagonal weight -> psum [128=(ri*64+dn*4+i), (c,b)].
  tanh:  tanh(x+iy) = (2T + i*(1-T^2)*sin2y) / D,
         D = (1+T^2) + (1-T^2)*cos2y = 2*(1 - q*u), q=1-T^2, u=sin^2(y).
         T=Tanh(x), s=Sin(2y), c=Sin(y) — all in ACT's accurate range.
  wilson: gather h1 rows k-major; 3 DVE passes of full-row products.
  omega: gather h2 rows; PE transpose -> comps on partitions; 5 accumulating
         matmuls (K=128) vs complex-expanded weights; tanh; free-axis reduce.
"""
import math

import numpy as np

import concourse.bacc as bacc
import concourse.bass as bass
import concourse.mybir as mybir
import concourse.tile as tile
from concourse import ap_utils
from concourse.bass_utils import run_bass_kernel_spmd
from concourse.masks import make_identity

AFT = mybir.ActivationFunctionType
ALU = mybir.AluOpType
F32 = mybir.dt.float32
I16 = mybir.dt.int16

B, N_SITES, N_PLAQ = 128, 16384, 16384
K_CHI, P_SZ, K_OMG = 9, 4, 5
C_CHI, C_OMG = 4, 4
WILSON_RESCALE = 10 ** 1.5
NCORES = 8
BL = B // NCORES
DEBUG_DUMP = False

# ---- chi (site-sharded, full batch) ----
DN = 14                     # sites per partition-group (14*9=126 partitions)
CHI_COLS = 4                # column-groups per chunk -> N = 4*128 = 512
CHI_SITES = DN * CHI_COLS   # 56 sites per chunk
S_LOC = N_SITES // NCORES   # 2048 real sites per core
CHI_NCH = (S_LOC + CHI_SITES - 1) // CHI_SITES      # 37
S_PAD = CHI_NCH * CHI_SITES                         # 2072 rows per a2a block

# ---- wilson / omega (batch-sharded) ----
NQ = 4                      # a2a sub-chunks
SQ = S_PAD // NQ            # 518 sites per sub-chunk per core
WIL_PC = 256                # plaquettes per gather call (4*256 = 1024 idx)
WIL_NCH = N_PLAQ // WIL_PC
OMG_PC = 1024               # plaquettes per chunk; 5 calls of 1024 idx
OMG_NCH = N_PLAQ // OMG_PC


def _wrap_idx16(flat):
    n = len(flat)
    a = flat.reshape(n // 16, 16).T
    return np.tile(a, (8, 1)).astype(np.int16)


def dma_gather_small(gp, out_ap, in_ap, idxs_ap, num_idxs, elem_size, elem_step):
    """bass dma_gather (DRAM src, non-transpose) without the 256B elem-size
    restriction (row stride must still be a 256B multiple)."""
    from concourse.bass import exact_div, round_up_to_multiple
    assert idxs_ap.dtype == mybir.dt.int16
    assert in_ap.space == bass.MemorySpace.DRAM
    assert out_ap.space == bass.MemorySpace.SBUF
    assert ap_utils.ap_is_contiguous(in_ap.ap[1:])
    assert ap_utils.ap_is_contiguous(out_ap.ap[1:])
    assert ap_utils.ap_is_contiguous(idxs_ap.ap[1:])
    assert out_ap.ap[-1][1] == elem_size
    assert out_ap.ap[0][1] * out_ap.ap[1][1] == round_up_to_multiple(num_idxs, 128)
    assert in_ap.ap[0][0] == elem_step
    stride_bytes_256 = exact_div(elem_step * mybir.dt.size(in_ap.dtype), 256)
    _in_ap = gp.lower_ap_dma(in_ap, for_custom_bir_dma=True)
    return gp.add_instruction(
        mybir.InstDMAGatherAnt(
            name=gp.bass.get_next_instruction_name(),
            ins=[*_in_ap, gp.lower_ap(idxs_ap),
                 gp.lower_val_access(gp.to_reg(num_idxs))],
            outs=[gp.lower_ap(out_ap)],
            transpose=False, num_idxs=num_idxs, elem_size=elem_size,
            stride_bytes_256=stride_bytes_256, gen_mode=0, single_packet=True,
            queue_num=0, sbuf_tokens_per_rank=0, sbuf_free_dim_per_rank=0,
            sbuf_free_dim_pad_per_rank=0, sbuf_byte_offset=0,
        )
    )


def build_host_tables(chi_kernel_idx, plaquette_idx, omega_kernel_idx,
                      chi_w, omega_w):
    # ---- per-core chi gather tables: core cc handles sites
    # [cc*2048, (cc+1)*2048); local padded to 2072 (pads gather row N_SITES).
    ci = np.concatenate(
        [chi_kernel_idx, np.full((CHI_SITES, K_CHI), N_SITES, np.int64)])
    chi_gidx_cores = []
    j = np.arange(CHI_NCH * CHI_COLS * 128)
    col = j // 128
    p = j % 128
    dn = p // K_CHI
    k = p % K_CHI
    nl = col * DN + np.minimum(dn, DN - 1)          # local site 0..2071
    for cc in range(NCORES):
        n = np.where(nl < S_LOC, cc * S_LOC + nl, N_SITES)
        flat = ci[np.minimum(n, N_SITES), k]
        flat[p >= DN * K_CHI] = 0
        chi_gidx_cores.append(_wrap_idx16(flat))

    # ---- wilson gather (h1 lives in the a2a recv buffer: site n ->
    # row (n//2048)*2072 + n%2048)
    def h1row(n):
        src = n // S_LOC
        loc = n % S_LOC
        return (loc // SQ) * (NCORES * SQ) + src * SQ + (loc % SQ)
    flat = np.zeros(WIL_NCH * 4 * WIL_PC, np.int64)
    for ch in range(WIL_NCH):
        base = ch * WIL_PC
        jj = np.arange(4 * WIL_PC)
        kk = jj // WIL_PC
        cc_ = (jj % WIL_PC) // 128
        qq = jj % 128
        flat[ch * 4 * WIL_PC + jj] = h1row(
            plaquette_idx[base + cc_ * 128 + qq, kk])
    wil_gidx = _wrap_idx16(flat)

    # ---- omega gather
    flat = np.zeros(OMG_NCH * K_OMG * OMG_PC, np.int64)
    pos = 0
    for ch in range(OMG_NCH):
        base = ch * OMG_PC
        for k in range(K_OMG):
            flat[pos:pos + OMG_PC] = omega_kernel_idx[base:base + OMG_PC, k]
            pos += OMG_PC
    omg_gidx = _wrap_idx16(flat)

    # ---- chi weight lhsT [128, 128]: rows p=(dn*9+k), cols ri*64+dn*4+i
    wchi = np.zeros((128, 128), np.float32)
    for dn_ in range(DN):
        for k in range(K_CHI):
            for i in range(C_CHI):
                wchi[dn_ * K_CHI + k, 0 * 64 + dn_ * 4 + i] = chi_w[i, 0, k].real
                wchi[dn_ * K_CHI + k, 1 * 64 + dn_ * 4 + i] = chi_w[i, 0, k].imag

    # ---- omega weights [128, 5*128]
    woms = []
    for k in range(K_OMG):
        w = np.zeros((128, 128), np.float32)
        for b in range(BL):
            for i in range(C_CHI):
                for o in range(C_OMG):
                    wr = omega_w[o, i, k].real
                    wi = omega_w[o, i, k].imag
                    w[0 * 64 + i * 16 + b, 0 * 64 + b * 4 + o] = wr
                    w[1 * 64 + i * 16 + b, 0 * 64 + b * 4 + o] = -wi
                    w[0 * 64 + i * 16 + b, 1 * 64 + b * 4 + o] = wi
                    w[1 * 64 + i * 16 + b, 1 * 64 + b * 4 + o] = wr
        woms.append(w)
    wom = np.concatenate(woms, axis=1)
    return chi_gidx_cores, wil_gidx, omg_gidx, wchi, wom


def emit_ctanh9_stacked(nc, pool, pslist, out_re, out_im, name):
    """Complex tanh; stacks len(pslist) psum-half pairs into the partition dim
    via ACT 1-input ops (which may shift partition base), then 9 DVE ops.

    pslist: list of (psum_x[H,F], psum_y[H,F]) APs; out_re/out_im [sum(H), F]
    SBUF APs at partition base 0.
    """
    H = pslist[0][0].shape[0]
    P = out_re.shape[0]
    F = out_re.free_size()

    def t(nm):
        return pool.tile([P, F], F32, name=f"{name}_{nm}", tag=f"{name}_{nm}",
                         bufs=2)
    T_, s_, c_, t2, q_, u_, d_, r_ = (t(x) for x in
                                      ("T", "s", "c", "t2", "q", "u", "d", "r"))
    for ui, (sx, sy) in enumerate(pslist):
        sl = slice(ui * 64, ui * 64 + H)
        nc.scalar.activation(T_[sl, :], sx, AFT.Tanh)
        nc.scalar.activation(s_[sl, :], sy, AFT.Sin, scale=2.0)
        nc.scalar.activation(c_[sl, :], sy, AFT.Sin)
    nc.vector.tensor_mul(t2[:], T_[:], T_[:])
    nc.vector.tensor_scalar(out=q_[:], in0=t2[:], scalar1=-1.0, scalar2=1.0,
                            op0=ALU.mult, op1=ALU.add)
    nc.vector.tensor_mul(u_[:], c_[:], c_[:])
    nc.vector.tensor_mul(u_[:], u_[:], q_[:])
    nc.vector.tensor_scalar(out=d_[:], in0=u_[:], scalar1=-2.0, scalar2=2.0,
                            op0=ALU.mult, op1=ALU.add)
    nc.vector.reciprocal(r_[:], d_[:])
    nc.vector.scalar_tensor_tensor(out=out_re, in0=T_[:], scalar=2.0,
                                   in1=r_[:], op0=ALU.mult, op1=ALU.mult)
    nc.vector.tensor_mul(s_[:], s_[:], q_[:])
    nc.vector.tensor_mul(out_im, s_[:], r_[:])


def build_kernel():
    nc = bacc.Bacc("TRN2", target_bir_lowering=False, debug=True)

    d_xf = nc.dram_tensor("xf", [N_SITES + 1, 128], F32, kind="ExternalInput")
    d_cgi = nc.dram_tensor("cgi", [128, CHI_NCH * CHI_COLS * 8], I16,
                           kind="ExternalInput")
    d_wgi = nc.dram_tensor("wgi", [128, WIL_NCH * 4 * WIL_PC // 16], I16,
                           kind="ExternalInput")
    d_ogi = nc.dram_tensor("ogi", [128, OMG_NCH * K_OMG * OMG_PC // 16], I16,
                           kind="ExternalInput")
    d_wchi = nc.dram_tensor("wchi", [128, 128], F32, kind="ExternalInput")
    d_wom = nc.dram_tensor("wom", [128, K_OMG * 128], F32, kind="ExternalInput")
    dbg = "ExternalOutput" if DEBUG_DUMP else "Internal"
    d_sendq = [[nc.dram_tensor(f"sendbuf{q}_{ri}", [NCORES * SQ, 64], F32)
                for ri in range(2)] for q in range(NQ)]
    d_recvq = [[nc.dram_tensor(f"recvbuf{q}_{ri}", [NCORES * SQ, 64], F32)
                for ri in range(2)] for q in range(NQ)]
    d_h1 = nc.dram_tensor("h1", [NCORES * S_PAD, 128], F32, kind=dbg)
    d_h2 = nc.dram_tensor("h2", [N_PLAQ, 128], F32, kind=dbg)
    d_out = nc.dram_tensor("out", [128, 2], F32, kind="ExternalOutput")

    with tile.TileContext(nc) as tc:
        with tc.tile_pool(name="pidx", bufs=1) as pidx, \
             tc.tile_pool(name="pwork", bufs=1) as pool, \
             tc.tile_pool(name="ppsum", bufs=2, space="PSUM") as ppsum:
            t_cgi = pidx.tile([128, CHI_NCH * CHI_COLS * 8], I16, name="t_cgi")
            t_wgi = pidx.tile([128, WIL_NCH * 4 * WIL_PC // 16], I16,
                              name="t_wgi")
            t_ogi = pidx.tile([128, OMG_NCH * K_OMG * OMG_PC // 16], I16,
                              name="t_ogi")
            t_wchi = pidx.tile([128, 128], F32, name="t_wchi")
            t_wom = pidx.tile([128, K_OMG * 128], F32, name="t_wom")
            ident = pidx.tile([128, 128], F32, name="ident")
            nc.sync.dma_start(t_cgi[:], d_cgi[:])
            nc.sync.dma_start(t_wgi[:], d_wgi[:])
            nc.sync.dma_start(t_ogi[:], d_ogi[:])
            nc.sync.dma_start(t_wchi[:], d_wchi[:])
            nc.sync.dma_start(t_wom[:], d_wom[:])
            make_identity(nc, ident[:])

            # =========== chi (site-sharded, full batch) ===========
            for pr in range(0, CHI_NCH, 2):
                npair = min(2, CHI_NCH - pr)
                pss = []
                for u in range(npair):
                    ch = pr + u
                    g = pool.tile([128, CHI_COLS, 128], F32, name="gchi",
                                  tag="gchi", bufs=3)
                    dma_gather_small(
                        nc.gpsimd, g[:], d_xf[:],
                        t_cgi[:, ch * CHI_COLS * 8:(ch + 1) * CHI_COLS * 8],
                        CHI_COLS * 128, 128, 128)
                    pch = ppsum.tile([128, 512], F32, name="pchi", tag="pchi",
                                     bufs=2)
                    nc.tensor.matmul(
                        pch[:], lhsT=t_wchi[:],
                        rhs=g[:].rearrange("p a b -> p (a b)"),
                        start=True, stop=True)
                    pss.append(pch)
                P = 64 * (npair - 1) + 56
                h1re = pool.tile([120, 512], F32, name="h1re", tag="h1re",
                                 bufs=2)
                h1im = pool.tile([120, 512], F32, name="h1im", tag="h1im",
                                 bufs=2)
                emit_ctanh9_stacked(
                    nc, pool,
                    [(p_[0:56, :], p_[64:120, :]) for p_ in pss],
                    h1re[0:P, :], h1im[0:P, :], "ctchi")
                # store to sendbuf rows (bg*S_PAD + local site)
                for u in range(npair):
                    nbase = (pr + u) * CHI_SITES
                    for (ri, tl) in ((0, h1re), (1, h1im)):
                        src4 = tl[:, :].rearrange(
                            "p (c b) -> p c b", c=CHI_COLS)
                        for c_ in range(CHI_COLS):
                            site0 = nbase + c_ * DN
                            q = site0 // SQ
                            lrow = site0 % SQ
                            # dst rows lrow..lrow+14, comps i*16+b with
                            # partition p=(dn*4+i) -> offset 16*p
                            dst = d_sendq[q][ri].rearrange(
                                "(bg s) f -> bg s f", bg=NCORES)[
                                :, lrow:lrow + DN, :].rearrange(
                                "bg dn (i b) -> (dn i) bg b", i=C_CHI)
                            eng = nc.scalar if (ri == 1 and c_ >= 2) \
                                else nc.sync
                            eng.dma_start(
                                dst,
                                src4[u * 64:u * 64 + 56, c_, :]
                                .rearrange("p (bg b) -> p bg b", bg=8))
            # ====== chunked all-to-all (re/im) + merge into h1 rows ======
            for q in range(NQ):
                for ri in range(2):
                    nc.gpsimd.collective_compute(
                        "AllToAll", ALU.bypass,
                        replica_groups=[list(range(NCORES))],
                        ins=[d_sendq[q][ri][:]], outs=[d_recvq[q][ri][:]])
                    nc.sync.dma_start(
                        d_h1[q * NCORES * SQ:(q + 1) * NCORES * SQ,
                             ri * 64:ri * 64 + 64],
                        d_recvq[q][ri][:])
            # =========== wilson ===========
            for ch in range(WIL_NCH):
                g = pool.tile([128, 8, 128], F32, name="gwil", tag="gwil",
                              bufs=3)
                dma_gather_small(
                    nc.gpsimd, g[:], d_h1[:],
                    t_wgi[:, ch * (4 * WIL_PC // 16):
                          (ch + 1) * (4 * WIL_PC // 16)],
                    4 * WIL_PC, 128, 128)
                m1 = pool.tile([128, 2, 128], F32, name="wm1", tag="wm1",
                               bufs=2)
                h2t = pool.tile([128, 2, 128], F32, name="h2t", tag="h2t",
                                bufs=2)
                nc.vector.tensor_mul(m1[:], g[:, 0:2, :], g[:, 2:4, :])
                nc.vector.tensor_mul(h2t[:], g[:, 4:6, :], g[:, 6:8, :])
                nc.vector.scalar_tensor_tensor(
                    out=h2t[:], in0=m1[:], scalar=float(WILSON_RESCALE),
                    in1=h2t[:], op0=ALU.mult, op1=ALU.mult)
                nc.sync.dma_start(
                    d_h2[ch * WIL_PC:(ch + 1) * WIL_PC, :]
                    .rearrange("(c q) f -> q c f", c=2), h2t[:])
            # =========== omega ===========
            acc_re = pool.tile([64, 1], F32, name="acc_re")
            acc_im = pool.tile([64, 1], F32, name="acc_im")
            nc.vector.memset(acc_re[:], 0.0)
            nc.vector.memset(acc_im[:], 0.0)
            for ch in range(OMG_NCH):
                gk = []
                for k in range(K_OMG):
                    gko = pool.tile([128, 8, 128], F32, name="gomg",
                                    tag=f"gomg{k}", bufs=2)
                    off = (ch * K_OMG + k) * (OMG_PC // 16)
                    dma_gather_small(nc.gpsimd, gko[:], d_h2[:],
                                     t_ogi[:, off:off + OMG_PC // 16],
                                     OMG_PC, 128, 128)
                    gk.append(gko)
                poms = []
                for half in range(2):
                    pom = ppsum.tile([128, 512], F32, name="pom", tag="pom",
                                     bufs=2)
                    for cb in range(4):
                        c = half * 4 + cb
                        ptr = ppsum.tile([128, 512], F32, name="ptr",
                                         tag="ptr", bufs=2)
                        gT = pool.tile([128, 512], F32, name="gT4", tag="gT4",
                                       bufs=2)
                        gT5 = pool.tile([128, 128], F32, name="gT5", tag="gT5",
                                        bufs=2)
                        for k in range(4):
                            nc.tensor.transpose(
                                ptr[:, k * 128:(k + 1) * 128],
                                gk[k][:, c, :], ident[:])
                        nc.vector.tensor_copy(out=gT[:], in_=ptr[:])
                        ptr5 = ppsum.tile([128, 128], F32, name="ptr5",
                                          tag="ptr5", bufs=2)
                        nc.tensor.transpose(ptr5[:], gk[4][:, c, :], ident[:])
                        nc.vector.tensor_copy(out=gT5[:], in_=ptr5[:])
                        for k in range(K_OMG):
                            rhs = gT[:, k * 128:(k + 1) * 128] if k < 4 \
                                else gT5[:]
                            nc.tensor.matmul(
                                pom[:, cb * 128:(cb + 1) * 128],
                                lhsT=t_wom[:, k * 128:(k + 1) * 128],
                                rhs=rhs, start=(k == 0), stop=(k == K_OMG - 1))
                    poms.append(pom)
                h3re = pool.tile([128, 512], F32, name="h3re", tag="h3re",
                                 bufs=2)
                h3im = pool.tile([128, 512], F32, name="h3im", tag="h3im",
                                 bufs=2)
                emit_ctanh9_stacked(
                    nc, pool,
                    [(p_[0:64, :], p_[64:128, :]) for p_ in poms],
                    h3re[:], h3im[:], "ctom")
                pre = pool.tile([128, 1], F32, name="pre", tag="pre", bufs=2)
                pim = pool.tile([128, 1], F32, name="pim", tag="pim", bufs=2)
                nc.vector.tensor_reduce(out=pre[:], in_=h3re[:],
                                        axis=mybir.AxisListType.X, op=ALU.add)
                nc.vector.tensor_reduce(out=pim[:], in_=h3im[:],
                                        axis=mybir.AxisListType.X, op=ALU.add)
                nc.vector.tensor_add(acc_re[:], acc_re[:], pre[0:64, :])
                nc.vector.tensor_add(acc_im[:], acc_im[:], pim[0:64, :])
                pre2 = pool.tile([64, 1], F32, name="pre2", tag="pre2", bufs=2)
                pim2 = pool.tile([64, 1], F32, name="pim2", tag="pim2", bufs=2)
                nc.vector.tensor_copy(out=pre2[:], in_=pre[64:128, :])
                nc.vector.tensor_copy(out=pim2[:], in_=pim[64:128, :])
                nc.vector.tensor_add(acc_re[:], acc_re[:], pre2[:])
                nc.vector.tensor_add(acc_im[:], acc_im[:], pim2[:])
            out_t = pool.tile([64, 2], F32, name="out_t")
            nc.vector.tensor_copy(out=out_t[:, 0:1], in_=acc_re[:])
            nc.vector.tensor_copy(out=out_t[:, 1:2], in_=acc_im[:])
            nc.sync.dma_start(d_out[0:64, :], out_t[:])
    nc.compile()
    return nc


_NC_CACHE = None


def kernel(x, chi_kernel_idx, chi_kernel_mask, plaquette_idx, plaquette_mask,
           omega_kernel_idx, omega_kernel_mask, chi_w, chi_b, omega_w,
           omega_b, _want_trace=False):
    global _NC_CACHE
    x = np.asarray(x, np.float32)
    chi_kernel_idx = np.asarray(chi_kernel_idx).astype(np.int64)
    plaquette_idx = np.asarray(plaquette_idx).astype(np.int64)
    omega_kernel_idx = np.asarray(omega_kernel_idx).astype(np.int64)
    chi_w = np.asarray(chi_w)
    omega_w = np.asarray(omega_w)

    chi_gidx_cores, wil_gidx, omg_gidx, wchi, wom = build_host_tables(
        chi_kernel_idx, plaquette_idx, omega_kernel_idx, chi_w, omega_w)

    if _NC_CACHE is None:
        _NC_CACHE = build_kernel()
    nc = _NC_CACHE

    xf = np.zeros((N_SITES + 1, 128), np.float32)
    xf[:N_SITES] = x.T
    in_maps = []
    for c in range(NCORES):
        in_maps.append({
            "xf": xf, "cgi": chi_gidx_cores[c], "wgi": wil_gidx,
            "ogi": omg_gidx, "wchi": wchi, "wom": wom,
        })
    r = run_bass_kernel_spmd(nc, in_maps, core_ids=list(range(NCORES)),
                             trace=_want_trace)
    out = np.zeros(B, np.complex64)
    for c in range(NCORES):
        o = r.results[c]["out"]
        v = o[:64, 0].reshape(BL, C_OMG).sum(1) + \
            1j * o[:64, 1].reshape(BL, C_OMG).sum(1)
        out[c * BL:(c + 1) * BL] = v.astype(np.complex64)
    if _want_trace:
        kernel._last_result = r
    return out

